# revision 19
# baseline (speedup 1.0000x reference)
"""MoE experts FFN kernel for Trainium2 (8 NeuronCores, expert parallel).

Reference computation (per expert e of 8):
    inter = hidden_states[e] @ gate_up_w[e]        # (C,H)@(H,2I) -> (C,2I)
    gate, up = split(inter, 2, axis=-1)
    act = silu(gate) * up                          # (C,I)
    out[e] = act @ down_w[e]                       # (C,I)@(I,H) -> (C,H)

E == n_cores == 8, so each core owns one expert end-to-end (no collectives).

Device-side layout trick: the PE computes lhsT.T @ rhs with the contraction
dim on partitions for BOTH operands.  Feeding x transposed (Xt = x.T, [H,C])
lets every matmul use naturally-laid-out weights as the stationary operand
and produces transposed intermediates:

    interT[f, c] = sum_h Wgu[h, f] * Xt[h, c]      (lhsT = Wgu tile, rhs = Xt)
    actT          = silu(gateT) * upT              (elementwise, any layout)
    outT[h, c]   = sum_i Wd[i, h] * actT[i, c]     (lhsT = Wd tile, rhs = actT)

The host transposes x on the way in and outT on the way out, casts inputs to
fp16 (fp32 accumulate in PSUM keeps the contraction accurate), and pre-packs
the weights into per-output-block tiles -- [block, p, kt, f] -- so every
weight-group load is one fully-contiguous DMA with 4KB-per-partition runs.

Schedule notes (the matmul stream itself runs at the fp16 PE roofline of
~216ns per 128x128x512 matmul; everything below is about the ~20us of
non-stream time):
  * Phase 2 opens with a double-block pass: i in {0,1} x c in {0,1}
    kt-outer across all 8 PSUM banks, then the same for c in {2,3}.
    Covering two i-blocks per xt k-tile halves the early xt HBM demand
    (a single-block first pass needs ~300GB/s of xt supply, over the
    ~358GB/s per-core HBM limit once weight traffic is added).
  * The first two i-blocks' weights load as kt-chunk DMAs on the Sync
    HWDGE ring in exact consumption order; the two kt=0 xt chunks ride
    the otherwise-idle Scalar HWDGE ring in parallel.  Scalar gets ONLY
    those two DMAs -- a long scalar DMA queue blocks the glu sigmoids at
    pass boundaries (measured -10us) because DMA issue paces at transfer
    completion rate.
  * A few dummy matmuls on a zeroed tile bridge the DMA bring-up window
    and start the HAM clock-gate warmup (1.2 -> 2.4 GHz after ~3.4us of
    sustained PE activity).
  * Each gate/up pass ends with a mirrored per-(i,c) bracket over the
    last two k-tiles so the first group's glu (which frees the PSUM bank
    the next pass's first matmul needs) completes well before the
    boundary.
  * The last output group is split into 4x N=128 sub-groups so the final
    PSUM->SBUF cast + output DMA tail after the last matmul is ~1us
    shorter.
"""

from contextlib import ExitStack

import numpy as np
import ml_dtypes

E, C, H, I = 8, 2048, 2048, 2816
F2 = 2 * I          # fused gate+up columns
P = 128             # partitions
NF = 512            # matmul moving free dim == one PSUM bank of fp32
KT = H // P         # 16 k-tiles over H (matmul 1)
IT = I // P         # 22 i-tiles over I
HT = H // P         # 16 h-tiles over H (matmul 2 output)
FT = F2 // P        # 44 f-blocks (22 gate + 22 up)
CT = C // NF        # 4 c-chunks

_NC_CACHE = {}


def _build_nc(compute="bfloat16"):
    if compute in _NC_CACHE:
        return _NC_CACHE[compute]

    import concourse.bacc as bacc
    import concourse.tile as tile
    from concourse import mybir

    cdt = getattr(mybir.dt, compute)
    f32 = mybir.dt.float32
    AFT = mybir.ActivationFunctionType

    nc = bacc.Bacc(None, target_bir_lowering=False, name="moe_expert_ffn")

    # wgu/wd arrive pre-packed: [block, p, kt* , f] with (kt, f) contiguous
    # per partition p (see make_in_maps).
    xt_d = nc.dram_tensor("xt", [H, C], cdt, kind="ExternalInput")
    wgu_d = nc.dram_tensor("wgu", [FT, P, KT, P], cdt, kind="ExternalInput")
    # first two i-blocks' gate/up weights pair-packed [i, 2, P, KT, P] so one
    # DMA carries a (g, u) chunk pair -- bigger transfers amortize the
    # per-DMA fixed cost during the bring-up crunch
    wab_d = nc.dram_tensor("wguab", [2, 2, P, KT, P], cdt, kind="ExternalInput")
    wd_d = nc.dram_tensor("wd", [HT, P, IT, P], cdt, kind="ExternalInput")
    odt = cdt if compute == "float16" else f32
    outT_d = nc.dram_tensor("outT", [H, C], odt, kind="ExternalOutput")

    xt_r = xt_d.ap().rearrange("(kt p) c -> p kt c", p=P)       # [128, KT, C]
    wgu_a = wgu_d.ap()
    wab_r = wab_d.ap().rearrange("i two p kt f -> i p two kt f")
    wd_a = wd_d.ap()
    outT_a = outT_d.ap()

    with tile.TileContext(nc) as tc, ExitStack() as ctx:
        singles = ctx.enter_context(tc.tile_pool(name="singles", bufs=1))
        wpool = ctx.enter_context(tc.tile_pool(name="wpool", bufs=2))
        tpool = ctx.enter_context(tc.tile_pool(name="tpool", bufs=2))
        opool = ctx.enter_context(tc.tile_pool(name="opool", bufs=3))
        psum = ctx.enter_context(tc.tile_pool(name="psum", bufs=4, space="PSUM"))

        def glu(i, c, g_ps, u_ps):
            c0 = c * NF
            s_sb = tpool.tile([P, NF], f32, tag="sig", name=f"sig{i}_{c}")
            nc.scalar.activation(out=s_sb, in_=g_ps, func=AFT.Sigmoid)
            nc.vector.tensor_mul(s_sb, g_ps, s_sb)
            nc.vector.tensor_mul(act_sb[i][:, c0 : c0 + NF], s_sb, u_ps)

        # Dummy matmuls on a zeroed tile bridge the PE's dead window while
        # the first inputs stream in and start the HAM clock warmup.
        xz = singles.tile([P, 256], cdt, tag="xz", name="xz")
        nc.vector.memset(xz, 0.0)
        warm_ps = psum.tile([P, 256], f32, tag="gps", name="warm_ps")
        for w in range(7):
            nc.tensor.matmul(warm_ps, xz[:, 0:P], xz, start=True, stop=True)

        # ---- critical-path loads ----
        GUK = 4            # kt per weight chunk

        # wab[i] holds the pair-packed (g, u) weights for i-block i.
        wab = [
            singles.tile([P, 2, KT, P], cdt, tag=f"wab{i}", name=f"wab{i}")
            for i in range(2)
        ]

        xt0_sb = []
        for c in range(CT):
            t = singles.tile([P, NF], cdt, tag=f"xt0_{c}", name=f"xt0_{c}")
            xt0_sb.append(t)

        def load_wab(i, k0, k1):
            nc.sync.dma_start(
                out=wab[i][:, :, k0:k1, :], in_=wab_r[i][:, :, k0:k1, :]
            )

        # xt k-tiles: kt 1..3 individually (latency-critical for the staged
        # head); kt 4..15 in three 4-tile groups so their column-half loads
        # are single 1MB DMAs (~341GB/s vs ~250 for 256KB).
        xt_sb = [None] + [
            singles.tile([P, C], cdt, tag=f"xt{kt}", name=f"xt{kt}")
            for kt in range(1, 4)
        ]
        xtg = [
            singles.tile([P, 4, C], cdt, tag=f"xtg{g}", name=f"xtg{g}")
            for g in range(3)
        ]

        # The two kt=0 xt chunks ride the Scalar HWDGE ring (the second,
        # otherwise-idle hardware DGE ring) so they stream in parallel with
        # the weight chunks on the Sync ring.
        nc.scalar.dma_start(out=xt0_sb[0], in_=xt_r[:, 0, 0:NF])
        nc.scalar.dma_start(out=xt0_sb[1], in_=xt_r[:, 0, NF : 2 * NF])

        # Sync ring, in just-in-time consumption order (arrival estimates at
        # ~250-341GB/s effective): i0 kt0-chunk pair, xt1/xt2 halves, i1
        # kt0 pair, xt3, then the kt4-7 / kt8-15 weight pairs and the 1MB
        # xt group halves.
        def load_xt_h1(kt):
            nc.sync.dma_start(
                out=xt_sb[kt][:, 0 : 2 * NF], in_=xt_r[:, kt, 0 : 2 * NF]
            )

        def load_xtg(g, half):
            c0, c1 = (0, 2 * NF) if half == 0 else (2 * NF, C)
            nc.sync.dma_start(
                out=xtg[g][:, :, c0:c1],
                in_=xt_r[:, 4 + 4 * g : 8 + 4 * g, c0:c1],
            )

        load_wab(0, 0, GUK)
        load_xt_h1(1)
        load_xt_h1(2)
        load_wab(1, 0, GUK)
        load_xt_h1(3)
        load_wab(0, GUK, 2 * GUK)
        load_wab(1, GUK, 2 * GUK)
        load_xtg(0, 0)
        load_wab(0, 2 * GUK, KT)
        load_wab(1, 2 * GUK, KT)
        load_xtg(1, 0)
        load_xtg(2, 0)

        # Pass B inputs: xt0 c2/c3 + the second column halves.
        nc.sync.dma_start(out=xt0_sb[2], in_=xt_r[:, 0, 2 * NF : 3 * NF])
        nc.sync.dma_start(out=xt0_sb[3], in_=xt_r[:, 0, 3 * NF : 4 * NF])
        for kt in (1, 2, 3):
            nc.sync.dma_start(
                out=xt_sb[kt][:, 2 * NF :], in_=xt_r[:, kt, 2 * NF :]
            )
        for g in range(3):
            load_xtg(g, 1)

        def xt_ap(kt, c):
            if kt == 0:
                return xt0_sb[c][:, :]
            if kt < 4:
                return xt_sb[kt][:, c * NF : (c + 1) * NF]
            g, j = (kt - 4) // 4, (kt - 4) % 4
            return xtg[g][:, j, c * NF : (c + 1) * NF]

        # actT resident in SBUF, one tile per i-tile.
        act_sb = [
            singles.tile([P, C], cdt, tag=f"act{i}", name=f"act{i}")
            for i in range(IT)
        ]

        # ---- phase 2: interT = Wgu.T @ Xt, actT = silu(gateT)*upT ----
        def gu_pass(cpair, staged=False):
            g_ps = {}
            u_ps = {}
            for i in range(2):
                for c in cpair:
                    g_ps[i, c] = psum.tile(
                        [P, NF], f32, tag="gps", name=f"gps{i}_{c}"
                    )
                    u_ps[i, c] = psum.tile(
                        [P, NF], f32, tag="ups", name=f"ups{i}_{c}"
                    )

            def mm(i, c, kt):
                for gu, ps in ((0, g_ps[i, c]), (1, u_ps[i, c])):
                    nc.tensor.matmul(
                        ps,
                        wab[i][:, gu, kt, :],
                        xt_ap(kt, c),
                        start=(kt == 0),
                        stop=(kt == KT - 1),
                    )

            # Staged head (pass A only): walk kt 0..GUK-1 one (i, c) bracket
            # at a time, in the order the first DMA chunks arrive.  The PSUM
            # accumulation groups don't care that i1's early k-tiles run
            # after i0's -- each group still opens at kt=0 / closes at kt=15.
            kt0 = 0
            if staged:
                kt0 = GUK
                for i in range(2):
                    for c in cpair:
                        for kt in range(GUK):
                            mm(i, c, kt)
            # Mirrored tail: the last two k-tiles run one (i, c) bracket at
            # a time, so group (0, c0) closes ~3us before the pass ends and
            # its glu (which frees the PSUM bank the NEXT pass's first
            # matmul needs) is long done at the boundary.
            for kt in range(kt0, KT - 2):
                for c in cpair:
                    for i in range(2):
                        mm(i, c, kt)
            for i in range(2):
                for c in cpair:
                    for kt in (KT - 2, KT - 1):
                        mm(i, c, kt)
            for i in range(2):
                for c in cpair:
                    glu(i, c, g_ps[i, c], u_ps[i, c])

        gu_pass((0, 1), staged=True)
        gu_pass((2, 3))

        def load_gu_weights(i, bufs=2):
            wg = wpool.tile(
                [P, KT, P], cdt, tag="wg", name=f"wg{i}", bufs=bufs
            )
            wu = wpool.tile(
                [P, KT, P], cdt, tag="wu", name=f"wu{i}", bufs=bufs
            )
            nc.sync.dma_start(out=wg, in_=wgu_a[i])
            nc.sync.dma_start(out=wu, in_=wgu_a[IT + i])
            return wg, wu

        for i in range(2, IT):
            wg, wu = load_gu_weights(i)
            for c in range(CT):
                g_ps = psum.tile([P, NF], f32, tag="gps", name=f"gps{i}_{c}")
                u_ps = psum.tile([P, NF], f32, tag="ups", name=f"ups{i}_{c}")
                for kt in range(KT):
                    nc.tensor.matmul(
                        g_ps,
                        wg[:, kt, :],
                        xt_ap(kt, c),
                        start=(kt == 0),
                        stop=(kt == KT - 1),
                    )
                for kt in range(KT):
                    nc.tensor.matmul(
                        u_ps,
                        wu[:, kt, :],
                        xt_ap(kt, c),
                        start=(kt == 0),
                        stop=(kt == KT - 1),
                    )
                glu(i, c, g_ps, u_ps)

        # ---- phase 3: outT = Wd.T @ actT ----
        NTAIL = 4          # N=128 sub-groups for the very last output chunk
        for h in range(HT):
            wd_t = wpool.tile([P, IT, P], cdt, tag="wd", name=f"wd{h}")
            h0 = h * P
            nc.sync.dma_start(out=wd_t, in_=wd_a[h])
            for c in range(CT):
                c0 = c * NF
                last = h == HT - 1 and c == CT - 1
                # reuse phase-2 bank groups (8 banks total; no room for a
                # third tag)
                if not last:
                    o_ps = psum.tile(
                        [P, NF], f32, tag="gps" if c % 2 == 0 else "ups",
                        name=f"ops{h}_{c}",
                    )
                    for it in range(IT):
                        nc.tensor.matmul(
                            o_ps,
                            wd_t[:, it, :],
                            act_sb[it][:, c0 : c0 + NF],
                            start=(it == 0),
                            stop=(it == IT - 1),
                        )
                    o_sb = opool.tile(
                        [P, NF], odt, tag="osb", name=f"osb{h}_{c}"
                    )
                    nc.vector.tensor_copy(out=o_sb, in_=o_ps)
                    nc.sync.dma_start(
                        out=outT_a[h0 : h0 + P, c0 : c0 + NF], in_=o_sb
                    )
                else:
                    # final chunk in N=128 slices: the after-last-matmul
                    # tail is one small cast + one 32KB DMA instead of a
                    # full 512-col cast + 128KB DMA.
                    NS = NF // NTAIL
                    for s in range(NTAIL):
                        s0 = c0 + s * NS
                        o_ps = psum.tile(
                            [P, NS], f32, tag="gps" if s % 2 == 0 else "ups",
                            name=f"opsF{s}",
                        )
                        for it in range(IT):
                            nc.tensor.matmul(
                                o_ps,
                                wd_t[:, it, :],
                                act_sb[it][:, s0 : s0 + NS],
                                start=(it == 0),
                                stop=(it == IT - 1),
                            )
                        o_sb = opool.tile(
                            [P, NS], odt, tag="osbF", name=f"osbF{s}"
                        )
                        nc.vector.tensor_copy(out=o_sb, in_=o_ps)
                        nc.sync.dma_start(
                            out=outT_a[h0 : h0 + P, s0 : s0 + NS], in_=o_sb
                        )

    nc.compile()
    _NC_CACHE[compute] = nc
    return nc


def _np_dtype(compute):
    return {"bfloat16": ml_dtypes.bfloat16, "float16": np.float16, "float32r": np.float32}[compute]


def _pack_w(w, n_k, n_b):
    """[K, B*P] -> [B, P, n_k, P] with (kt, f) contiguous per partition p."""
    return np.ascontiguousarray(
        w.reshape(n_k, P, n_b, P).transpose(2, 1, 0, 3)
    )


def make_in_maps(hidden_states, gate_up_w, down_w, compute="bfloat16"):
    dt = _np_dtype(compute)
    in_maps = []
    for e in range(E):
        wgu = _pack_w(gate_up_w[e].astype(dt), KT, FT)
        # pair-packed (g, u) copy of the first two i-blocks: [i, 2, P, KT, P]
        wguab = np.ascontiguousarray(
            np.stack([wgu[0:2], wgu[IT : IT + 2]], axis=1)
        )
        in_maps.append(
            {
                "xt": np.ascontiguousarray(hidden_states[e].T).astype(dt),
                "wgu": wgu,
                "wguab": wguab,
                "wd": _pack_w(down_w[e].astype(dt), IT, HT),
            }
        )
    return in_maps


def run_hw(in_maps, compute="bfloat16", trace=False, **kwargs):
    from concourse import bass_utils

    if trace:
        # local-only devloop: skip the artifact-bucket upload
        bass_utils.upload_artifacts = lambda tmpdir: f"local:{tmpdir}"
    nc = _build_nc(compute)
    return bass_utils.run_bass_kernel_spmd(
        nc, in_maps, core_ids=list(range(E)), trace=trace, **kwargs
    )


def kernel(hidden_states, gate_up_w, down_w):
    compute = "float16"
    hidden_states = np.asarray(hidden_states)
    gate_up_w = np.asarray(gate_up_w)
    down_w = np.asarray(down_w)
    in_maps = make_in_maps(hidden_states, gate_up_w, down_w, compute)
    res = run_hw(in_maps, compute)
    out = np.empty((E, C, H), dtype=np.float32)
    for e in range(E):
        out[e] = res.results[e]["outT"].T
    return out
